# revision 26
# baseline (speedup 1.0000x reference)
"""Trainium2 Bass kernel for a 2-layer masked LSTM + FC + sigmoid head.

Problem shapes (hardcoded): B=1024, T=512, I=16, H=64.
Sharding: pure data parallel, batch 1024 -> 8 cores x 128.

Per-core design (v3 — fused-K recurrent matmuls, one sigmoid per step)
----------------------------------------------------------------------
PSUM: two [128, 1024] tiles (2 banks each) rotate over step PAIRS. The
512-col region of super-step k holds [IF0 | OG0 | IF1 | OG1] blocks
(128 cols of batch each): layer 0 at t=k, layer 1 at tau=k-4.

Layer 0 needs NO separate input projection: gates = [W_hh0; b0; W_ih0]
@ [h0(t-1); 1; x(t)] as ONE K=81 matmul per gate-block. The rhs lives
in an 8-slot staging ring [81, 3072] (slot = [zeros(128) | h0+ones+x
(128) | zeros(128)], stride 384): the h-write lands h0(t) in slot t%8
rows 0:64, x(t+1) is DMA'd into rows 65:81, row 64 is constant 1. The
IF matmul takes rhs [stg|0] and writes [IF0|OG0] cols (zero half
accumulates +0 into OG0); the OG matmul takes [0|stg] into the same
range. Moving dim is 256 both times.

Layer 1: input projection W_ih1 @ h0 + b1 is batched per step-pair
(strided N=256 matmuls with a ones-row bias rider, lhsT [65, 128])
into the IF1/OG1 columns, emitted 2 steps before use so it never waits
on the critical chain; the recurrent K=64 matmuls accumulate on top
from an h1 ring [64, 2176] (8 slots x [zero|h1], stride 256).

One sigmoid ACT [128, 512] per step covers i, f, 2g, o of BOTH layers:
tanh(g) = 2*sigmoid(2g) - 1 with the 2x folded into the g weights.
DVE trio (layer-packed [64, 256] via strided APs):
  fc = f*c;  ig2 = (sig2g - 0.5)*i;  c' = 2*ig2 + fc
then tanh(c') and two h-write mults. Layer-1's matmuls are emitted
ahead of layer-0's each step so the chain is: h0w -> IF0-MM -> OG0-MM
-> sigmoid -> fc -> ig2 -> c' -> tanh -> h0w.

Masked final-state capture: run unmasked; h2 += d_t * h1(t) with
one-hot d[b,t] = mask[b,t] - mask[b,t+1] host-prebroadcast to
dbc[64, T*128], DMA-streamed; the multiply-accumulate runs on the
otherwise-idle GpSimd engine as two [64, 512] ops per 4-step block.
"""

from contextlib import ExitStack

import numpy as np

import concourse.bass as bass
import concourse.tile as tile
from concourse import bacc, mybir
from concourse import bass_utils

F32 = mybir.dt.float32
F32R = mybir.dt.float32r
AF = mybir.ActivationFunctionType
OP = mybir.AluOpType

B, T, I, H = 1024, 512, 16, 64
NCORES = 8
BL = B // NCORES  # 128 batch per core
LAG = 4           # layer-1 step lag

_BUILT = {}


def _build_program(t_steps: int):
    nc = bacc.Bacc(
        "TRN2",
        target_bir_lowering=False,
        debug=False,
        enable_asserts=False,
        num_devices=NCORES,
    )

    TB = t_steps * BL  # 65536
    d_xs = nc.dram_tensor("xs", [16, TB], F32R, kind="ExternalInput")
    d_dbc = nc.dram_tensor("dbc", [64, TB], F32, kind="ExternalInput")
    wnames = [
        ("wif0", 81), ("wog0", 81),      # [W_hh0; b0; W_ih0] fused lhsT
        ("w0if1", 65), ("w0og1", 65),    # [W_ih1; b1] rider lhsT
        ("whif1", 64), ("whog1", 64),    # W_hh1 lhsT
    ]
    d_w = {}
    for name, k in wnames:
        d_w[name] = nc.dram_tensor(name, [k, 128], F32R, kind="ExternalInput")
    d_fct = nc.dram_tensor("fct", [64, 1], F32R, kind="ExternalInput")
    d_fcb = nc.dram_tensor("fcb", [1, 1], F32, kind="ExternalInput")
    d_out = nc.dram_tensor("out", [1, 128], F32, kind="ExternalOutput")

    NCH = TB // 2048  # 32 dbc chunks of 16 steps

    with tile.TileContext(nc) as tc, ExitStack() as ctx:
        pconst = ctx.enter_context(tc.tile_pool(name="const", bufs=1))
        pstate = ctx.enter_context(tc.tile_pool(name="state", bufs=1))
        ppsum = ctx.enter_context(tc.tile_pool(name="psum", bufs=1, space="PSUM"))
        pwork = ctx.enter_context(tc.tile_pool(name="work", bufs=3))

        # ---- weights ----
        w = {}
        for name, k in wnames:
            w[name] = pconst.tile([k, 128], F32R, tag=name, name=name)
            nc.sync.dma_start(w[name][:], d_w[name].ap()[:])
        fct = pconst.tile([64, 1], F32R, tag="fct")
        nc.sync.dma_start(fct[:], d_fct.ap()[:])
        fcb = pconst.tile([1, 1], F32, tag="fcb")
        nc.sync.dma_start(fcb[:], d_fcb.ap()[:])

        # ---- rings ----
        # ring0 slot s (stride 384): [zero(128) | h0 rows 0:64, ones row 64,
        # x rows 65:81 (128 cols) | zero(128)]
        ring0 = pstate.tile([81, 8 * 384], F32R, tag="ring0")
        nc.vector.memset(ring0[:].bitcast(F32), 0.0)
        ones_sl = ring0[64:65, :].rearrange("p (s c) -> p s c", c=384)[:, :, 128:256]
        nc.vector.memset(ones_sl.bitcast(F32), 1.0)
        # ring1 slot s (stride 256): [zero(128) | h1(128)]; +128 pad cols
        ring1 = pstate.tile([64, 8 * 256 + 128], F32R, tag="ring1")
        nc.vector.memset(ring1[:].bitcast(F32), 0.0)

        dcb = [pconst.tile([64, 2048], F32, tag=f"dcb{i}", name=f"dcb{i}")
               for i in range(3)]
        for j in range(2):
            nc.sync.dma_start(dcb[j][:], d_dbc.ap()[:, j * 2048:(j + 1) * 2048])

        def dma_x(t0, n):
            # x(t0..t0+n-1) -> ring0 slots (t0-1)%8 .. contiguous, rows 65:81
            s0 = (t0 - 1) % 8
            dst = ring0[65:81, 384 * s0:384 * (s0 + n)]
            dst = dst.rearrange("p (s c) -> p s c", c=384)[:, :, 128:256]
            nc.sync.dma_start(dst, d_xs.ap()[:, t0 * BL:(t0 + n) * BL])

        dma_x(0, 1)   # x(0) -> slot 7
        dma_x(1, 4)   # slots 0..3
        dma_x(5, 4)   # slots 4..7 (overwrites nothing: slot7 x(0) read at k=0
                      # before x(8)'s DMA lands; tile deps serialize)

        c_sb = pstate.tile([128, 256], F32, tag="csb")
        nc.vector.memset(c_sb[:], 0.0)
        h2acc = pstate.tile([64, 512], F32, tag="h2acc")
        nc.vector.memset(h2acc[:], 0.0)

        # ---- PSUM: per-layer step-pair banks (no cross-layer group
        # interleaving within a bank: start/stop sequences stay sequential)
        pg0 = [ppsum.tile([128, 512], F32, tag=f"pg0{i}", name=f"pg0{i}")
               for i in range(2)]
        pg1 = [ppsum.tile([128, 512], F32, tag=f"pg1{i}", name=f"pg1{i}")
               for i in range(2)]

        mm = nc.tensor.matmul

        for k in range(t_steps + LAG):
            t = k
            tau = k - LAG
            l0 = t < t_steps
            l1 = 0 <= tau < t_steps
            bk0 = pg0[(k // 2) % 2]
            s0c = 256 * (k % 2)
            bk1 = pg1[((k - LAG) // 2) % 2] if k >= LAG else None
            s1c = 256 * ((k - LAG) % 2) if k >= LAG else 0

            # ---- layer-0 fused matmuls (chain leader) ----
            if l0:
                out = bk0[:, s0c:s0c + 256]
                sp = (t - 1) % 8
                rif = ring0[0:81, 384 * sp + 128:384 * sp + 384]   # [stg | 0]
                rog = ring0[0:81, 384 * sp:384 * sp + 256]         # [0 | stg]
                mm(out, w["wif0"][:], rif, start=True, stop=False,
                   skip_group_check=True)
                mm(out, w["wog0"][:], rog, start=False, stop=True,
                   skip_group_check=True)

            # ---- layer-1 recurrent matmuls (trail layer 0 by a stage) ----
            if l1 and tau >= 1:
                out = bk1[:, s1c:s1c + 256]
                sp = (tau - 1) % 8
                rif = ring1[0:64, 256 * sp + 128:256 * sp + 384]   # [h1 | 0]
                rog = ring1[0:64, 256 * sp:256 * sp + 256]         # [0 | h1]
                mm(out, w["whif1"][:], rif, start=False, stop=False,
                   skip_group_check=True)
                mm(out, w["whog1"][:], rog, start=False, stop=(tau % 2 == 1),
                   skip_group_check=True)

            # ---- per-layer sigmoid + cell update (decoupled chains;
            # layer-0's tanh/h-write emitted before layer-1's trio so the
            # critical chain never queues behind off-chain work) ----
            g_sb = pwork.tile([128, 512], F32, tag="gsb")
            tc_sb = pwork.tile([128, 256], F32, tag="tcsb")
            fco = pwork.tile([64, 256], F32, tag="fco")
            ig2 = pwork.tile([64, 256], F32, tag="ig2")
            if l0:
                # split sigmoid: IF half overlaps the OG matmul, so fc can
                # start while the OG half is still activating
                nc.scalar.activation(g_sb[:, 0:128], bk0[:, s0c:s0c + 128],
                                     AF.Sigmoid)
                nc.scalar.activation(g_sb[:, 128:256], bk0[:, s0c + 128:s0c + 256],
                                     AF.Sigmoid)
            if l1:
                nc.scalar.activation(g_sb[:, 256:512], bk1[:, s1c:s1c + 256],
                                     AF.Sigmoid)

            def trio(ell):
                b0c = 256 * ell
                co = 128 * ell
                cin = c_sb[64:128, co:co + 128]
                nc.vector.tensor_tensor(fco[:, co:co + 128],
                                        g_sb[64:128, b0c:b0c + 128], cin, OP.mult)
                nc.vector.scalar_tensor_tensor(ig2[:, co:co + 128],
                                               g_sb[0:64, b0c + 128:b0c + 256], 0.5,
                                               g_sb[0:64, b0c:b0c + 128],
                                               OP.subtract, OP.mult)
                nc.vector.scalar_tensor_tensor(cin, ig2[:, co:co + 128], 2.0,
                                               fco[:, co:co + 128], OP.mult, OP.add)

            if l0:
                with tc.high_priority():
                    trio(0)
                    nc.scalar.activation(tc_sb[64:128, 0:128], c_sb[64:128, 0:128],
                                         AF.Tanh)
                    dst = ring0[0:64, 384 * (t % 8) + 128:384 * (t % 8) + 256]
                    nc.vector.tensor_tensor(dst, g_sb[64:128, 128:256],
                                            tc_sb[64:128, 0:128], OP.mult)
            if l1:
                # layer-1 trio routed through tc_sb: fc1 overwrites the tanh0
                # slot, whose WAR on h0w's read forces the DVE to run the
                # chain-critical h0w BEFORE layer-1's cell update
                cin1 = c_sb[64:128, 128:256]
                fc1o = tc_sb[64:128, 0:128]
                ig1o = tc_sb[64:128, 128:256]
                nc.vector.tensor_tensor(fc1o, g_sb[64:128, 256:384], cin1, OP.mult)
                nc.vector.scalar_tensor_tensor(ig1o, g_sb[0:64, 384:512], 0.5,
                                               g_sb[0:64, 256:384],
                                               OP.subtract, OP.mult)
                nc.vector.scalar_tensor_tensor(cin1, ig1o, 2.0, fc1o,
                                               OP.mult, OP.add)
                nc.scalar.activation(tc_sb[64:128, 128:256], cin1, AF.Tanh)
                dst = ring1[0:64, 256 * (tau % 8) + 128:256 * (tau % 8) + 256]
                nc.vector.tensor_tensor(dst, g_sb[64:128, 384:512],
                                        tc_sb[64:128, 128:256], OP.mult)

            # ---- capture on GpSimd, once per 4-step block ----
            if k % 4 == 3 and k >= LAG + 3:
                c = (k - 3 - LAG) // 4
                t0 = 4 * c
                ch = dcb[(t0 // 16) % 3]
                dsl = ch[:, (t0 % 16) * 128:(t0 % 16) * 128 + 512]
                sp = t0 % 8
                h1s = ring1[0:64, 256 * sp:256 * (sp + 4)]
                h1s = h1s.rearrange("p (s c) -> p s c", c=256)[:, :, 128:256]
                mblk = pwork.tile([64, 512], F32, tag="mblk")
                nc.vector.tensor_tensor(mblk[:], dsl, h1s, OP.mult)
                nc.vector.tensor_tensor(h2acc[:], h2acc[:], mblk[:], OP.add)

            # ---- layer-1 input projection, 1.5 steps ahead of use ----
            if k % 2 == 1:
                q = (k - 3) // 2
                if 0 <= q < t_steps // 2:
                    tb = pg1[q % 2]
                    rr = tb[:].rearrange("p (s c) -> p s c", c=256)
                    t0 = 2 * q
                    sp = t0 % 8
                    rhs = ring0[0:65, 384 * sp:384 * (sp + 2)]
                    rhs = rhs.rearrange("p (s c) -> p s c", c=384)[:, :, 128:256]
                    mm(rr[:, :, 0:128], w["w0if1"][:], rhs, start=True,
                       stop=False, skip_group_check=True)
                    mm(rr[:, :, 128:256], w["w0og1"][:], rhs, start=True,
                       stop=False, skip_group_check=True)

            # ---- input streaming ----
            if k % 4 == 0 and k >= 8 and k + 1 < t_steps:
                n = min(4, t_steps - (k + 1))
                dma_x(k + 1, n)
            if k % 16 == 4 and k // 16 + 2 < NCH:
                j = k // 16 + 2
                nc.sync.dma_start(dcb[j % 3][:], d_dbc.ap()[:, j * 2048:(j + 1) * 2048])

        # ---------- FC + sigmoid head ----------
        hfold = pwork.tile([64, 256], F32, tag="hfold")
        nc.vector.tensor_tensor(hfold[:], h2acc[:, 0:256], h2acc[:, 256:512], OP.add)
        h2 = pwork.tile([64, 128], F32R, tag="h2")
        nc.vector.tensor_tensor(h2[:], hfold[:, 0:128], hfold[:, 128:256], OP.add)
        mm(pg0[0][0:1, 0:128], fct[:], h2[:], start=True, stop=True,
           skip_group_check=True)
        osb = pwork.tile([1, 128], F32, tag="osb")
        nc.scalar.activation(osb[:], pg0[0][0:1, 0:128], AF.Sigmoid, bias=fcb[:, 0:1])
        nc.sync.dma_start(d_out.ap()[:], osb[:])

    nc.compile()
    return nc


def _get_program(t_steps: int):
    if t_steps not in _BUILT:
        _BUILT[t_steps] = _build_program(t_steps)
    return _BUILT[t_steps]


def _prep_core_inputs(x, dmask, weights, t_steps):
    """Host-side layout prep for one core's shard. x: [BL, T, I], dmask: [BL, T]."""
    TB = t_steps * BL
    xs = np.ascontiguousarray(
        np.asarray(x, np.float32).transpose(2, 1, 0).reshape(16, TB))
    dbc = np.ascontiguousarray(
        np.broadcast_to(dmask.T.reshape(1, TB), (64, TB)).astype(np.float32))
    return dict(xs=xs, dbc=dbc, **weights)


def _host_weights(w_ih0, w_hh0, b_ih0, b_hh0,
                  w_ih1, w_hh1, b_ih1, b_hh1, fc_w, fc_b):
    b0 = np.asarray(b_ih0, np.float32) + np.asarray(b_hh0, np.float32)
    b1 = np.asarray(b_ih1, np.float32) + np.asarray(b_hh1, np.float32)
    wih0, whh0 = np.asarray(w_ih0, np.float32), np.asarray(w_hh0, np.float32)
    wih1, whh1 = np.asarray(w_ih1, np.float32), np.asarray(w_hh1, np.float32)

    def og_w(wm):  # [4H, K] -> [2g; o] stacked [128, K] (PyTorch i,f,g,o rows)
        return np.concatenate([2.0 * wm[2 * H:3 * H], wm[3 * H:4 * H]], axis=0)

    def og_b(bv):
        return np.concatenate([2.0 * bv[2 * H:3 * H], bv[3 * H:4 * H]])

    def fused0(wx, wh, bv):  # [W_hh; b; W_ih] lhsT [81, 128]
        out = np.empty((81, 128), np.float32)
        out[0:64] = wh.T
        out[64] = bv
        out[65:81] = wx.T
        return out

    def rider1(wx, bv):  # [W_ih1; b1] lhsT [65, 128]
        out = np.empty((65, 128), np.float32)
        out[0:64] = wx.T
        out[64] = bv
        return out

    weights = dict(
        wif0=fused0(wih0[0:2 * H], whh0[0:2 * H], b0[0:2 * H]),
        wog0=fused0(og_w(wih0), og_w(whh0), og_b(b0)),
        w0if1=rider1(wih1[0:2 * H], b1[0:2 * H]),
        w0og1=rider1(og_w(wih1), og_b(b1)),
        whif1=np.ascontiguousarray(whh1[0:2 * H].T),
        whog1=np.ascontiguousarray(og_w(whh1).T),
        fct=np.ascontiguousarray(np.asarray(fc_w, np.float32).reshape(1, H).T),
        fcb=np.asarray(fc_b, np.float32).reshape(1, 1),
    )
    return weights


def _run(x, mask, w_ih0, w_hh0, b_ih0, b_hh0,
         w_ih1, w_hh1, b_ih1, b_hh1, fc_w, fc_b, trace=False):
    t_steps = x.shape[1]
    x = np.asarray(x, np.float32)
    mask = np.asarray(mask)

    # d[b, t] = mask[b, t] - mask[b, t+1]  (one-hot at t = len_b - 1)
    m = mask.astype(np.float32)
    d = m - np.concatenate([m[:, 1:], np.zeros((m.shape[0], 1), np.float32)], axis=1)

    weights = _host_weights(w_ih0, w_hh0, b_ih0, b_hh0,
                            w_ih1, w_hh1, b_ih1, b_hh1, fc_w, fc_b)

    nc = _get_program(t_steps)
    in_maps = []
    for c in range(NCORES):
        sl = slice(c * BL, (c + 1) * BL)
        in_maps.append(_prep_core_inputs(x[sl], d[sl], weights, t_steps))

    res = bass_utils.run_bass_kernel_spmd(nc, in_maps, core_ids=list(range(NCORES)),
                                          trace=trace)
    out = np.concatenate([res.results[c]["out"].reshape(BL) for c in range(NCORES)])
    return out.astype(np.float32), res


def kernel(**inputs):
    return _run(**inputs)[0]


def kernel_traced(**inputs):
    return _run(**inputs, trace=True)


# revision 28
# speedup vs baseline: 1.0017x; 1.0017x over previous
"""Trainium2 Bass kernel for a 2-layer masked LSTM + FC + sigmoid head.

Problem shapes (hardcoded): B=1024, T=512, I=16, H=64.
Sharding: pure data parallel, batch 1024 -> 8 cores x 128.

Per-core design (v3 — fused-K recurrent matmuls, one sigmoid per step)
----------------------------------------------------------------------
PSUM: two [128, 1024] tiles (2 banks each) rotate over step PAIRS. The
512-col region of super-step k holds [IF0 | OG0 | IF1 | OG1] blocks
(128 cols of batch each): layer 0 at t=k, layer 1 at tau=k-4.

Layer 0 needs NO separate input projection: gates = [W_hh0; b0; W_ih0]
@ [h0(t-1); 1; x(t)] as ONE K=81 matmul per gate-block. The rhs lives
in an 8-slot staging ring [81, 3072] (slot = [zeros(128) | h0+ones+x
(128) | zeros(128)], stride 384): the h-write lands h0(t) in slot t%8
rows 0:64, x(t+1) is DMA'd into rows 65:81, row 64 is constant 1. The
IF matmul takes rhs [stg|0] and writes [IF0|OG0] cols (zero half
accumulates +0 into OG0); the OG matmul takes [0|stg] into the same
range. Moving dim is 256 both times.

Layer 1: input projection W_ih1 @ h0 + b1 is batched per step-pair
(strided N=256 matmuls with a ones-row bias rider, lhsT [65, 128])
into the IF1/OG1 columns, emitted 2 steps before use so it never waits
on the critical chain; the recurrent K=64 matmuls accumulate on top
from an h1 ring [64, 2176] (8 slots x [zero|h1], stride 256).

One sigmoid ACT [128, 512] per step covers i, f, 2g, o of BOTH layers:
tanh(g) = 2*sigmoid(2g) - 1 with the 2x folded into the g weights.
DVE trio (layer-packed [64, 256] via strided APs):
  fc = f*c;  ig2 = (sig2g - 0.5)*i;  c' = 2*ig2 + fc
then tanh(c') and two h-write mults. Layer-1's matmuls are emitted
ahead of layer-0's each step so the chain is: h0w -> IF0-MM -> OG0-MM
-> sigmoid -> fc -> ig2 -> c' -> tanh -> h0w.

Masked final-state capture: run unmasked; h2 += d_t * h1(t) with
one-hot d[b,t] = mask[b,t] - mask[b,t+1] host-prebroadcast to
dbc[64, T*128], DMA-streamed; the multiply-accumulate runs on the
otherwise-idle GpSimd engine as two [64, 512] ops per 4-step block.
"""

from contextlib import ExitStack

import numpy as np

import concourse.bass as bass
import concourse.tile as tile
from concourse import bacc, mybir
from concourse import bass_utils

F32 = mybir.dt.float32
F32R = mybir.dt.float32r
AF = mybir.ActivationFunctionType
OP = mybir.AluOpType

B, T, I, H = 1024, 512, 16, 64
NCORES = 8
BL = B // NCORES  # 128 batch per core
LAG = 4           # layer-1 step lag

_BUILT = {}


def _build_program(t_steps: int):
    nc = bacc.Bacc(
        "TRN2",
        target_bir_lowering=False,
        debug=False,
        enable_asserts=False,
        num_devices=NCORES,
    )

    TB = t_steps * BL  # 65536
    d_xs = nc.dram_tensor("xs", [16, TB], F32R, kind="ExternalInput")
    d_dbc = nc.dram_tensor("dbc", [64, TB], F32, kind="ExternalInput")
    wnames = [
        ("wif0", 81), ("wog0", 81),      # [W_hh0; b0; W_ih0] fused lhsT
        ("w0if1", 65), ("w0og1", 65),    # [W_ih1; b1] rider lhsT
        ("whif1", 64), ("whog1", 64),    # W_hh1 lhsT
    ]
    d_w = {}
    for name, k in wnames:
        d_w[name] = nc.dram_tensor(name, [k, 128], F32R, kind="ExternalInput")
    d_fct = nc.dram_tensor("fct", [64, 1], F32R, kind="ExternalInput")
    d_fcb = nc.dram_tensor("fcb", [1, 1], F32, kind="ExternalInput")
    d_out = nc.dram_tensor("out", [1, 128], F32, kind="ExternalOutput")

    NCH = TB // 2048  # 32 dbc chunks of 16 steps

    with tile.TileContext(nc) as tc, ExitStack() as ctx:
        pconst = ctx.enter_context(tc.tile_pool(name="const", bufs=1))
        pstate = ctx.enter_context(tc.tile_pool(name="state", bufs=1))
        ppsum = ctx.enter_context(tc.tile_pool(name="psum", bufs=1, space="PSUM"))
        pwork = ctx.enter_context(tc.tile_pool(name="work", bufs=3))

        # ---- weights ----
        w = {}
        for name, k in wnames:
            w[name] = pconst.tile([k, 128], F32R, tag=name, name=name)
            nc.sync.dma_start(w[name][:], d_w[name].ap()[:])
        fct = pconst.tile([64, 1], F32R, tag="fct")
        nc.sync.dma_start(fct[:], d_fct.ap()[:])
        fcb = pconst.tile([1, 1], F32, tag="fcb")
        nc.sync.dma_start(fcb[:], d_fcb.ap()[:])

        # ---- rings ----
        # ring0 slot s (stride 384): [zero(128) | h0 rows 0:64, ones row 64,
        # x rows 65:81 (128 cols) | zero(128)]
        ring0 = pstate.tile([81, 8 * 384], F32R, tag="ring0")
        nc.vector.memset(ring0[:].bitcast(F32), 0.0)
        ones_sl = ring0[64:65, :].rearrange("p (s c) -> p s c", c=384)[:, :, 128:256]
        nc.vector.memset(ones_sl.bitcast(F32), 1.0)
        # ring1 slot s (stride 256): [zero(128) | h1(128)]; +128 pad cols
        ring1 = pstate.tile([64, 8 * 256 + 128], F32R, tag="ring1")
        nc.vector.memset(ring1[:].bitcast(F32), 0.0)

        dcb = [pconst.tile([64, 2048], F32, tag=f"dcb{i}", name=f"dcb{i}")
               for i in range(3)]
        for j in range(2):
            nc.sync.dma_start(dcb[j][:], d_dbc.ap()[:, j * 2048:(j + 1) * 2048])

        def dma_x(t0, n):
            # x(t0..t0+n-1) -> ring0 slots (t0-1)%8 .. contiguous, rows 65:81
            s0 = (t0 - 1) % 8
            dst = ring0[65:81, 384 * s0:384 * (s0 + n)]
            dst = dst.rearrange("p (s c) -> p s c", c=384)[:, :, 128:256]
            nc.sync.dma_start(dst, d_xs.ap()[:, t0 * BL:(t0 + n) * BL])

        dma_x(0, 1)   # x(0) -> slot 7
        dma_x(1, 4)   # slots 0..3
        dma_x(5, 4)   # slots 4..7 (overwrites nothing: slot7 x(0) read at k=0
                      # before x(8)'s DMA lands; tile deps serialize)

        c_sb = pstate.tile([128, 256], F32, tag="csb")
        nc.vector.memset(c_sb[:], 0.0)
        h2acc = pstate.tile([64, 1024], F32, tag="h2acc")
        nc.vector.memset(h2acc[:], 0.0)

        # ---- PSUM: per-layer step-pair banks (no cross-layer group
        # interleaving within a bank: start/stop sequences stay sequential)
        pg0 = [ppsum.tile([128, 512], F32, tag=f"pg0{i}", name=f"pg0{i}")
               for i in range(2)]
        pg1 = [ppsum.tile([128, 512], F32, tag=f"pg1{i}", name=f"pg1{i}")
               for i in range(2)]

        mm = nc.tensor.matmul

        for k in range(t_steps + LAG):
            t = k
            tau = k - LAG
            l0 = t < t_steps
            l1 = 0 <= tau < t_steps
            bk0 = pg0[(k // 2) % 2]
            s0c = 256 * (k % 2)
            bk1 = pg1[((k - LAG) // 2) % 2] if k >= LAG else None
            s1c = 256 * ((k - LAG) % 2) if k >= LAG else 0

            # ---- layer-0 fused matmuls (chain leader) ----
            if l0:
                out = bk0[:, s0c:s0c + 256]
                sp = (t - 1) % 8
                rif = ring0[0:81, 384 * sp + 128:384 * sp + 384]   # [stg | 0]
                rog = ring0[0:81, 384 * sp:384 * sp + 256]         # [0 | stg]
                mm(out, w["wif0"][:], rif, start=True, stop=False,
                   skip_group_check=True)
                mm(out, w["wog0"][:], rog, start=False, stop=True,
                   skip_group_check=True)

            # ---- layer-1 recurrent matmuls (trail layer 0 by a stage) ----
            if l1 and tau >= 1:
                out = bk1[:, s1c:s1c + 256]
                sp = (tau - 1) % 8
                rif = ring1[0:64, 256 * sp + 128:256 * sp + 384]   # [h1 | 0]
                rog = ring1[0:64, 256 * sp:256 * sp + 256]         # [0 | h1]
                mm(out, w["whif1"][:], rif, start=False, stop=False,
                   skip_group_check=True)
                mm(out, w["whog1"][:], rog, start=False, stop=(tau % 2 == 1),
                   skip_group_check=True)

            # ---- per-layer sigmoid + cell update (decoupled chains;
            # layer-0's tanh/h-write emitted before layer-1's trio so the
            # critical chain never queues behind off-chain work) ----
            g_sb = pwork.tile([128, 512], F32, tag="gsb")
            tc_sb = pwork.tile([128, 256], F32, tag="tcsb")
            fco = pwork.tile([64, 256], F32, tag="fco")
            ig2 = pwork.tile([64, 256], F32, tag="ig2")
            if l0:
                # split sigmoid: IF half overlaps the OG matmul, so fc can
                # start while the OG half is still activating
                nc.scalar.activation(g_sb[:, 0:128], bk0[:, s0c:s0c + 128],
                                     AF.Sigmoid)
                nc.scalar.activation(g_sb[:, 128:256], bk0[:, s0c + 128:s0c + 256],
                                     AF.Sigmoid)
            if l1:
                nc.scalar.activation(g_sb[:, 256:512], bk1[:, s1c:s1c + 256],
                                     AF.Sigmoid)

            def trio(ell):
                b0c = 256 * ell
                co = 128 * ell
                cin = c_sb[64:128, co:co + 128]
                nc.vector.tensor_tensor(fco[:, co:co + 128],
                                        g_sb[64:128, b0c:b0c + 128], cin, OP.mult)
                nc.vector.scalar_tensor_tensor(ig2[:, co:co + 128],
                                               g_sb[0:64, b0c + 128:b0c + 256], 0.5,
                                               g_sb[0:64, b0c:b0c + 128],
                                               OP.subtract, OP.mult)
                nc.vector.scalar_tensor_tensor(cin, ig2[:, co:co + 128], 2.0,
                                               fco[:, co:co + 128], OP.mult, OP.add)

            if l0:
                with tc.high_priority():
                    trio(0)
                    nc.scalar.activation(tc_sb[64:128, 0:128], c_sb[64:128, 0:128],
                                         AF.Tanh)
                    dst = ring0[0:64, 384 * (t % 8) + 128:384 * (t % 8) + 256]
                    nc.vector.tensor_tensor(dst, g_sb[64:128, 128:256],
                                            tc_sb[64:128, 0:128], OP.mult)
            if l1:
                trio(1)
                nc.scalar.activation(tc_sb[64:128, 128:256], c_sb[64:128, 128:256],
                                     AF.Tanh)
                dst = ring1[0:64, 256 * (tau % 8) + 128:256 * (tau % 8) + 256]
                nc.vector.tensor_tensor(dst, g_sb[64:128, 384:512],
                                        tc_sb[64:128, 128:256], OP.mult)

            # ---- capture, once per 8-step block (full h1 ring) ----
            if k % 8 == 3 and k >= LAG + 7:
                c = (k - 7 - LAG) // 8
                t0 = 8 * c
                ch = dcb[(t0 // 16) % 3]
                dsl = ch[:, (t0 % 16) * 128:(t0 % 16) * 128 + 1024]
                h1s = ring1[0:64, 0:2048]
                h1s = h1s.rearrange("p (s c) -> p s c", c=256)[:, :, 128:256]
                mblk = pwork.tile([64, 1024], F32, tag="mblk")
                nc.vector.tensor_tensor(mblk[:], dsl, h1s, OP.mult)
                nc.vector.tensor_tensor(h2acc[:], h2acc[:], mblk[:], OP.add)

            # ---- layer-1 input projection, 1.5 steps ahead of use ----
            if k % 2 == 1:
                q = (k - 3) // 2
                if 0 <= q < t_steps // 2:
                    tb = pg1[q % 2]
                    rr = tb[:].rearrange("p (s c) -> p s c", c=256)
                    t0 = 2 * q
                    sp = t0 % 8
                    rhs = ring0[0:65, 384 * sp:384 * (sp + 2)]
                    rhs = rhs.rearrange("p (s c) -> p s c", c=384)[:, :, 128:256]
                    mm(rr[:, :, 0:128], w["w0if1"][:], rhs, start=True,
                       stop=False, skip_group_check=True)
                    mm(rr[:, :, 128:256], w["w0og1"][:], rhs, start=True,
                       stop=False, skip_group_check=True)

            # ---- input streaming ----
            if k % 4 == 0 and k >= 8 and k + 1 < t_steps:
                n = min(4, t_steps - (k + 1))
                dma_x(k + 1, n)
            if k % 16 == 4 and k // 16 + 2 < NCH:
                j = k // 16 + 2
                nc.sync.dma_start(dcb[j % 3][:], d_dbc.ap()[:, j * 2048:(j + 1) * 2048])

        # ---------- FC + sigmoid head ----------
        hfold2 = pwork.tile([64, 512], F32, tag="hfold2")
        nc.vector.tensor_tensor(hfold2[:], h2acc[:, 0:512], h2acc[:, 512:1024], OP.add)
        hfold = pwork.tile([64, 256], F32, tag="hfold")
        nc.vector.tensor_tensor(hfold[:], hfold2[:, 0:256], hfold2[:, 256:512], OP.add)
        h2 = pwork.tile([64, 128], F32R, tag="h2")
        nc.vector.tensor_tensor(h2[:], hfold[:, 0:128], hfold[:, 128:256], OP.add)
        mm(pg0[0][0:1, 0:128], fct[:], h2[:], start=True, stop=True,
           skip_group_check=True)
        osb = pwork.tile([1, 128], F32, tag="osb")
        nc.scalar.activation(osb[:], pg0[0][0:1, 0:128], AF.Sigmoid, bias=fcb[:, 0:1])
        nc.sync.dma_start(d_out.ap()[:], osb[:])

    nc.compile()
    return nc


def _get_program(t_steps: int):
    if t_steps not in _BUILT:
        _BUILT[t_steps] = _build_program(t_steps)
    return _BUILT[t_steps]


def _prep_core_inputs(x, dmask, weights, t_steps):
    """Host-side layout prep for one core's shard. x: [BL, T, I], dmask: [BL, T]."""
    TB = t_steps * BL
    xs = np.ascontiguousarray(
        np.asarray(x, np.float32).transpose(2, 1, 0).reshape(16, TB))
    dbc = np.ascontiguousarray(
        np.broadcast_to(dmask.T.reshape(1, TB), (64, TB)).astype(np.float32))
    return dict(xs=xs, dbc=dbc, **weights)


def _host_weights(w_ih0, w_hh0, b_ih0, b_hh0,
                  w_ih1, w_hh1, b_ih1, b_hh1, fc_w, fc_b):
    b0 = np.asarray(b_ih0, np.float32) + np.asarray(b_hh0, np.float32)
    b1 = np.asarray(b_ih1, np.float32) + np.asarray(b_hh1, np.float32)
    wih0, whh0 = np.asarray(w_ih0, np.float32), np.asarray(w_hh0, np.float32)
    wih1, whh1 = np.asarray(w_ih1, np.float32), np.asarray(w_hh1, np.float32)

    def og_w(wm):  # [4H, K] -> [2g; o] stacked [128, K] (PyTorch i,f,g,o rows)
        return np.concatenate([2.0 * wm[2 * H:3 * H], wm[3 * H:4 * H]], axis=0)

    def og_b(bv):
        return np.concatenate([2.0 * bv[2 * H:3 * H], bv[3 * H:4 * H]])

    def fused0(wx, wh, bv):  # [W_hh; b; W_ih] lhsT [81, 128]
        out = np.empty((81, 128), np.float32)
        out[0:64] = wh.T
        out[64] = bv
        out[65:81] = wx.T
        return out

    def rider1(wx, bv):  # [W_ih1; b1] lhsT [65, 128]
        out = np.empty((65, 128), np.float32)
        out[0:64] = wx.T
        out[64] = bv
        return out

    weights = dict(
        wif0=fused0(wih0[0:2 * H], whh0[0:2 * H], b0[0:2 * H]),
        wog0=fused0(og_w(wih0), og_w(whh0), og_b(b0)),
        w0if1=rider1(wih1[0:2 * H], b1[0:2 * H]),
        w0og1=rider1(og_w(wih1), og_b(b1)),
        whif1=np.ascontiguousarray(whh1[0:2 * H].T),
        whog1=np.ascontiguousarray(og_w(whh1).T),
        fct=np.ascontiguousarray(np.asarray(fc_w, np.float32).reshape(1, H).T),
        fcb=np.asarray(fc_b, np.float32).reshape(1, 1),
    )
    return weights


def _run(x, mask, w_ih0, w_hh0, b_ih0, b_hh0,
         w_ih1, w_hh1, b_ih1, b_hh1, fc_w, fc_b, trace=False):
    t_steps = x.shape[1]
    x = np.asarray(x, np.float32)
    mask = np.asarray(mask)

    # d[b, t] = mask[b, t] - mask[b, t+1]  (one-hot at t = len_b - 1)
    m = mask.astype(np.float32)
    d = m - np.concatenate([m[:, 1:], np.zeros((m.shape[0], 1), np.float32)], axis=1)

    weights = _host_weights(w_ih0, w_hh0, b_ih0, b_hh0,
                            w_ih1, w_hh1, b_ih1, b_hh1, fc_w, fc_b)

    nc = _get_program(t_steps)
    in_maps = []
    for c in range(NCORES):
        sl = slice(c * BL, (c + 1) * BL)
        in_maps.append(_prep_core_inputs(x[sl], d[sl], weights, t_steps))

    res = bass_utils.run_bass_kernel_spmd(nc, in_maps, core_ids=list(range(NCORES)),
                                          trace=trace)
    out = np.concatenate([res.results[c]["out"].reshape(BL) for c in range(NCORES)])
    return out.astype(np.float32), res


def kernel(**inputs):
    return _run(**inputs)[0]


def kernel_traced(**inputs):
    return _run(**inputs, trace=True)


# revision 30
# speedup vs baseline: 1.0720x; 1.0701x over previous
"""Trainium2 Bass kernel for a 2-layer masked LSTM + FC + sigmoid head.

Problem shapes (hardcoded): B=1024, T=512, I=16, H=64.
Sharding: pure data parallel, batch 1024 -> 8 cores x 128.

Per-core design (v3 — fused-K recurrent matmuls, one sigmoid per step)
----------------------------------------------------------------------
PSUM: two [128, 1024] tiles (2 banks each) rotate over step PAIRS. The
512-col region of super-step k holds [IF0 | OG0 | IF1 | OG1] blocks
(128 cols of batch each): layer 0 at t=k, layer 1 at tau=k-4.

Layer 0 needs NO separate input projection: gates = [W_hh0; b0; W_ih0]
@ [h0(t-1); 1; x(t)] as ONE K=81 matmul per gate-block. The rhs lives
in an 8-slot staging ring [81, 3072] (slot = [zeros(128) | h0+ones+x
(128) | zeros(128)], stride 384): the h-write lands h0(t) in slot t%8
rows 0:64, x(t+1) is DMA'd into rows 65:81, row 64 is constant 1. The
IF matmul takes rhs [stg|0] and writes [IF0|OG0] cols (zero half
accumulates +0 into OG0); the OG matmul takes [0|stg] into the same
range. Moving dim is 256 both times.

Layer 1: input projection W_ih1 @ h0 + b1 is batched per step-pair
(strided N=256 matmuls with a ones-row bias rider, lhsT [65, 128])
into the IF1/OG1 columns, emitted 2 steps before use so it never waits
on the critical chain; the recurrent K=64 matmuls accumulate on top
from an h1 ring [64, 2176] (8 slots x [zero|h1], stride 256).

One sigmoid ACT [128, 512] per step covers i, f, 2g, o of BOTH layers:
tanh(g) = 2*sigmoid(2g) - 1 with the 2x folded into the g weights.
DVE trio (layer-packed [64, 256] via strided APs):
  fc = f*c;  ig2 = (sig2g - 0.5)*i;  c' = 2*ig2 + fc
then tanh(c') and two h-write mults. Layer-1's matmuls are emitted
ahead of layer-0's each step so the chain is: h0w -> IF0-MM -> OG0-MM
-> sigmoid -> fc -> ig2 -> c' -> tanh -> h0w.

Masked final-state capture: run unmasked; h2 += d_t * h1(t) with
one-hot d[b,t] = mask[b,t] - mask[b,t+1] host-prebroadcast to
dbc[64, T*128], DMA-streamed; the multiply-accumulate runs on the
otherwise-idle GpSimd engine as two [64, 512] ops per 4-step block.
"""

from contextlib import ExitStack

import numpy as np

import concourse.bass as bass
import concourse.tile as tile
from concourse import bacc, mybir
from concourse import bass_utils

F32 = mybir.dt.float32
F32R = mybir.dt.float32r
AF = mybir.ActivationFunctionType
OP = mybir.AluOpType

B, T, I, H = 1024, 512, 16, 64
NCORES = 8
BL = B // NCORES  # 128 batch per core
LAG = 4           # layer-1 step lag

_BUILT = {}


def _build_program(t_steps: int):
    nc = bacc.Bacc(
        "TRN2",
        target_bir_lowering=False,
        debug=False,
        enable_asserts=False,
        num_devices=NCORES,
    )

    TB = t_steps * BL  # 65536
    d_xs = nc.dram_tensor("xs", [16, TB], F32R, kind="ExternalInput")
    d_dbc = nc.dram_tensor("dbc", [64, TB], F32, kind="ExternalInput")
    wnames = [
        ("wif0", 81), ("wog0", 81),      # [W_hh0; b0; W_ih0] fused lhsT
        ("w0if1", 65), ("w0og1", 65),    # [W_ih1; b1] rider lhsT
        ("whif1", 64), ("whog1", 64),    # W_hh1 lhsT
    ]
    d_w = {}
    for name, k in wnames:
        d_w[name] = nc.dram_tensor(name, [k, 128], F32R, kind="ExternalInput")
    d_fct = nc.dram_tensor("fct", [64, 1], F32R, kind="ExternalInput")
    d_fcb = nc.dram_tensor("fcb", [1, 1], F32, kind="ExternalInput")
    d_out = nc.dram_tensor("out", [1, 128], F32, kind="ExternalOutput")

    NCH = TB // 2048  # 32 dbc chunks of 16 steps

    with tile.TileContext(nc) as tc, ExitStack() as ctx:
        pconst = ctx.enter_context(tc.tile_pool(name="const", bufs=1))
        pstate = ctx.enter_context(tc.tile_pool(name="state", bufs=1))
        ppsum = ctx.enter_context(tc.tile_pool(name="psum", bufs=1, space="PSUM"))
        pwork = ctx.enter_context(tc.tile_pool(name="work", bufs=3))

        # ---- weights ----
        w = {}
        for name, k in wnames:
            w[name] = pconst.tile([k, 128], F32R, tag=name, name=name)
            nc.sync.dma_start(w[name][:], d_w[name].ap()[:])
        fct = pconst.tile([64, 1], F32R, tag="fct")
        nc.sync.dma_start(fct[:], d_fct.ap()[:])
        fcb = pconst.tile([1, 1], F32, tag="fcb")
        nc.sync.dma_start(fcb[:], d_fcb.ap()[:])

        # ---- rings ----
        # ring0 slot s (stride 384): [zero(128) | h0 rows 0:64, ones row 64,
        # x rows 65:81 (128 cols) | zero(128)]
        ring0 = pstate.tile([81, 8 * 384], F32R, tag="ring0")
        nc.vector.memset(ring0[:].bitcast(F32), 0.0)
        ones_sl = ring0[64:65, :].rearrange("p (s c) -> p s c", c=384)[:, :, 128:256]
        nc.vector.memset(ones_sl.bitcast(F32), 1.0)
        # ring1 slot s (stride 256): [zero(128) | h1(128)]; +128 pad cols
        ring1 = pstate.tile([64, 8 * 256 + 128], F32R, tag="ring1")
        nc.vector.memset(ring1[:].bitcast(F32), 0.0)

        dcb = [pconst.tile([64, 2048], F32, tag=f"dcb{i}", name=f"dcb{i}")
               for i in range(3)]
        for j in range(2):
            nc.sync.dma_start(dcb[j][:], d_dbc.ap()[:, j * 2048:(j + 1) * 2048])

        def dma_x(t0, n):
            # x(t0..t0+n-1) -> ring0 slots (t0-1)%8 .. contiguous, rows 65:81
            s0 = (t0 - 1) % 8
            dst = ring0[65:81, 384 * s0:384 * (s0 + n)]
            dst = dst.rearrange("p (s c) -> p s c", c=384)[:, :, 128:256]
            nc.sync.dma_start(dst, d_xs.ap()[:, t0 * BL:(t0 + n) * BL])

        dma_x(0, 1)   # x(0) -> slot 7
        dma_x(1, 4)   # slots 0..3
        dma_x(5, 4)   # slots 4..7 (overwrites nothing: slot7 x(0) read at k=0
                      # before x(8)'s DMA lands; tile deps serialize)

        c_sb = pstate.tile([128, 256], F32, tag="csb")
        nc.vector.memset(c_sb[:], 0.0)
        h2acc = pstate.tile([64, 512], F32, tag="h2acc")
        nc.vector.memset(h2acc[:], 0.0)

        # ---- PSUM: per-layer step-pair banks (no cross-layer group
        # interleaving within a bank: start/stop sequences stay sequential)
        pg0 = [ppsum.tile([128, 512], F32, tag=f"pg0{i}", name=f"pg0{i}")
               for i in range(2)]
        pg1 = [ppsum.tile([128, 512], F32, tag=f"pg1{i}", name=f"pg1{i}")
               for i in range(2)]

        mm = nc.tensor.matmul

        for k in range(t_steps + LAG):
            t = k
            tau = k - LAG
            l0 = t < t_steps
            l1 = 0 <= tau < t_steps
            bk0 = pg0[(k // 2) % 2]
            s0c = 256 * (k % 2)
            bk1 = pg1[((k - LAG) // 2) % 2] if k >= LAG else None
            s1c = 256 * ((k - LAG) % 2) if k >= LAG else 0

            # ---- layer-0 fused matmuls (chain leader) ----
            if l0:
                out = bk0[:, s0c:s0c + 256]
                sp = (t - 1) % 8
                rif = ring0[0:81, 384 * sp + 128:384 * sp + 384]   # [stg | 0]
                rog = ring0[0:81, 384 * sp:384 * sp + 256]         # [0 | stg]
                mm(out, w["wif0"][:], rif, start=True, stop=False,
                   skip_group_check=True)
                mm(out, w["wog0"][:], rog, start=False, stop=True,
                   skip_group_check=True)

            # ---- layer-1 recurrent matmuls (trail layer 0 by a stage) ----
            if l1 and tau >= 1:
                out = bk1[:, s1c:s1c + 256]
                sp = (tau - 1) % 8
                rif = ring1[0:64, 256 * sp + 128:256 * sp + 384]   # [h1 | 0]
                rog = ring1[0:64, 256 * sp:256 * sp + 256]         # [0 | h1]
                mm(out, w["whif1"][:], rif, start=False, stop=False,
                   skip_group_check=True)
                mm(out, w["whog1"][:], rog, start=False, stop=(tau % 2 == 1),
                   skip_group_check=True)

            # ---- per-layer sigmoid + cell update (decoupled chains;
            # layer-0's tanh/h-write emitted before layer-1's trio so the
            # critical chain never queues behind off-chain work) ----
            g_sb = pwork.tile([128, 512], F32, tag="gsb")
            tc_sb = pwork.tile([128, 256], F32, tag="tcsb")
            fco = pwork.tile([64, 256], F32, tag="fco")
            ig2 = pwork.tile([64, 256], F32, tag="ig2")
            if l0:
                # split sigmoid: IF half overlaps the OG matmul, so fc can
                # start while the OG half is still activating
                nc.scalar.activation(g_sb[:, 0:128], bk0[:, s0c:s0c + 128],
                                     AF.Sigmoid)
                nc.scalar.activation(g_sb[:, 128:256], bk0[:, s0c + 128:s0c + 256],
                                     AF.Sigmoid)
            if l1:
                nc.scalar.activation(g_sb[:, 256:384], bk1[:, s1c:s1c + 128],
                                     AF.Sigmoid)
                nc.scalar.activation(g_sb[:, 384:512], bk1[:, s1c + 128:s1c + 256],
                                     AF.Sigmoid)

            def trio(ell):
                b0c = 256 * ell
                co = 128 * ell
                cin = c_sb[64:128, co:co + 128]
                nc.vector.tensor_tensor(fco[:, co:co + 128],
                                        g_sb[64:128, b0c:b0c + 128], cin, OP.mult)
                nc.vector.scalar_tensor_tensor(ig2[:, co:co + 128],
                                               g_sb[0:64, b0c + 128:b0c + 256], 0.5,
                                               g_sb[0:64, b0c:b0c + 128],
                                               OP.subtract, OP.mult)
                nc.vector.scalar_tensor_tensor(cin, ig2[:, co:co + 128], 2.0,
                                               fco[:, co:co + 128], OP.mult, OP.add)

            if l0:
                with tc.high_priority():
                    trio(0)
                    nc.scalar.activation(tc_sb[64:128, 0:128], c_sb[64:128, 0:128],
                                         AF.Tanh)
                    dst = ring0[0:64, 384 * (t % 8) + 128:384 * (t % 8) + 256]
                    nc.vector.tensor_tensor(dst, g_sb[64:128, 128:256],
                                            tc_sb[64:128, 0:128], OP.mult)
            if l1:
                trio(1)
                nc.scalar.activation(tc_sb[64:128, 128:256], c_sb[64:128, 128:256],
                                     AF.Tanh)
                dst = ring1[0:64, 256 * (tau % 8) + 128:256 * (tau % 8) + 256]
                nc.vector.tensor_tensor(dst, g_sb[64:128, 384:512],
                                        tc_sb[64:128, 128:256], OP.mult)

            # ---- capture on GpSimd, once per 4-step block ----
            if k % 4 == 3 and k >= LAG + 3:
                c = (k - 3 - LAG) // 4
                t0 = 4 * c
                ch = dcb[(t0 // 16) % 3]
                dsl = ch[:, (t0 % 16) * 128:(t0 % 16) * 128 + 512]
                sp = t0 % 8
                h1s = ring1[0:64, 256 * sp:256 * (sp + 4)]
                h1s = h1s.rearrange("p (s c) -> p s c", c=256)[:, :, 128:256]
                mblk = pwork.tile([64, 512], F32, tag="mblk")
                nc.vector.tensor_tensor(mblk[:], dsl, h1s, OP.mult)
                nc.vector.tensor_tensor(h2acc[:], h2acc[:], mblk[:], OP.add)

            # ---- layer-1 input projection, 1.5 steps ahead of use ----
            if k % 2 == 1:
                q = (k - 3) // 2
                if 0 <= q < t_steps // 2:
                    tb = pg1[q % 2]
                    rr = tb[:].rearrange("p (s c) -> p s c", c=256)
                    t0 = 2 * q
                    sp = t0 % 8
                    rhs = ring0[0:65, 384 * sp:384 * (sp + 2)]
                    rhs = rhs.rearrange("p (s c) -> p s c", c=384)[:, :, 128:256]
                    mm(rr[:, :, 0:128], w["w0if1"][:], rhs, start=True,
                       stop=False, skip_group_check=True)
                    mm(rr[:, :, 128:256], w["w0og1"][:], rhs, start=True,
                       stop=False, skip_group_check=True)

            # ---- input streaming ----
            if k % 4 == 0 and k >= 8 and k + 1 < t_steps:
                n = min(4, t_steps - (k + 1))
                dma_x(k + 1, n)
            if k % 16 == 4 and k // 16 + 2 < NCH:
                j = k // 16 + 2
                nc.sync.dma_start(dcb[j % 3][:], d_dbc.ap()[:, j * 2048:(j + 1) * 2048])

        # ---------- FC + sigmoid head ----------
        hfold = pwork.tile([64, 256], F32, tag="hfold")
        nc.vector.tensor_tensor(hfold[:], h2acc[:, 0:256], h2acc[:, 256:512], OP.add)
        h2 = pwork.tile([64, 128], F32R, tag="h2")
        nc.vector.tensor_tensor(h2[:], hfold[:, 0:128], hfold[:, 128:256], OP.add)
        mm(pg0[0][0:1, 0:128], fct[:], h2[:], start=True, stop=True,
           skip_group_check=True)
        osb = pwork.tile([1, 128], F32, tag="osb")
        nc.scalar.activation(osb[:], pg0[0][0:1, 0:128], AF.Sigmoid, bias=fcb[:, 0:1])
        nc.sync.dma_start(d_out.ap()[:], osb[:])

    nc.compile()
    return nc


def _get_program(t_steps: int):
    if t_steps not in _BUILT:
        _BUILT[t_steps] = _build_program(t_steps)
    return _BUILT[t_steps]


def _prep_core_inputs(x, dmask, weights, t_steps):
    """Host-side layout prep for one core's shard. x: [BL, T, I], dmask: [BL, T]."""
    TB = t_steps * BL
    xs = np.ascontiguousarray(
        np.asarray(x, np.float32).transpose(2, 1, 0).reshape(16, TB))
    dbc = np.ascontiguousarray(
        np.broadcast_to(dmask.T.reshape(1, TB), (64, TB)).astype(np.float32))
    return dict(xs=xs, dbc=dbc, **weights)


def _host_weights(w_ih0, w_hh0, b_ih0, b_hh0,
                  w_ih1, w_hh1, b_ih1, b_hh1, fc_w, fc_b):
    b0 = np.asarray(b_ih0, np.float32) + np.asarray(b_hh0, np.float32)
    b1 = np.asarray(b_ih1, np.float32) + np.asarray(b_hh1, np.float32)
    wih0, whh0 = np.asarray(w_ih0, np.float32), np.asarray(w_hh0, np.float32)
    wih1, whh1 = np.asarray(w_ih1, np.float32), np.asarray(w_hh1, np.float32)

    def og_w(wm):  # [4H, K] -> [2g; o] stacked [128, K] (PyTorch i,f,g,o rows)
        return np.concatenate([2.0 * wm[2 * H:3 * H], wm[3 * H:4 * H]], axis=0)

    def og_b(bv):
        return np.concatenate([2.0 * bv[2 * H:3 * H], bv[3 * H:4 * H]])

    def fused0(wx, wh, bv):  # [W_hh; b; W_ih] lhsT [81, 128]
        out = np.empty((81, 128), np.float32)
        out[0:64] = wh.T
        out[64] = bv
        out[65:81] = wx.T
        return out

    def rider1(wx, bv):  # [W_ih1; b1] lhsT [65, 128]
        out = np.empty((65, 128), np.float32)
        out[0:64] = wx.T
        out[64] = bv
        return out

    weights = dict(
        wif0=fused0(wih0[0:2 * H], whh0[0:2 * H], b0[0:2 * H]),
        wog0=fused0(og_w(wih0), og_w(whh0), og_b(b0)),
        w0if1=rider1(wih1[0:2 * H], b1[0:2 * H]),
        w0og1=rider1(og_w(wih1), og_b(b1)),
        whif1=np.ascontiguousarray(whh1[0:2 * H].T),
        whog1=np.ascontiguousarray(og_w(whh1).T),
        fct=np.ascontiguousarray(np.asarray(fc_w, np.float32).reshape(1, H).T),
        fcb=np.asarray(fc_b, np.float32).reshape(1, 1),
    )
    return weights


def _run(x, mask, w_ih0, w_hh0, b_ih0, b_hh0,
         w_ih1, w_hh1, b_ih1, b_hh1, fc_w, fc_b, trace=False):
    t_steps = x.shape[1]
    x = np.asarray(x, np.float32)
    mask = np.asarray(mask)

    # d[b, t] = mask[b, t] - mask[b, t+1]  (one-hot at t = len_b - 1)
    m = mask.astype(np.float32)
    d = m - np.concatenate([m[:, 1:], np.zeros((m.shape[0], 1), np.float32)], axis=1)

    weights = _host_weights(w_ih0, w_hh0, b_ih0, b_hh0,
                            w_ih1, w_hh1, b_ih1, b_hh1, fc_w, fc_b)

    nc = _get_program(t_steps)
    in_maps = []
    for c in range(NCORES):
        sl = slice(c * BL, (c + 1) * BL)
        in_maps.append(_prep_core_inputs(x[sl], d[sl], weights, t_steps))

    res = bass_utils.run_bass_kernel_spmd(nc, in_maps, core_ids=list(range(NCORES)),
                                          trace=trace)
    out = np.concatenate([res.results[c]["out"].reshape(BL) for c in range(NCORES)])
    return out.astype(np.float32), res


def kernel(**inputs):
    return _run(**inputs)[0]


def kernel_traced(**inputs):
    return _run(**inputs, trace=True)
